# revision 100
# baseline (speedup 1.0000x reference)
"""BERT encoder (12 layers, B=8 T=512 D=768 H=12) on 8 Trainium2 NeuronCores.

Strategy: pure data parallelism — core b runs the full 12-layer stack for
batch element b. No collectives. All five per-layer GEMMs run on the tensor
engine in bf16 (fp32 PSUM accumulation); softmax uses ACT Exp; the softmax
normalization is applied to ctx on DVE; activation transposes come from PE
transpose-mode in bf16; layernorm runs fused on DVE/ACT in fp32.

The layer is software-pipelined to keep the PE's HAM clock warm (the PE
clock-gates to 1.2 GHz after any ~3.4us idle window and needs ~3.4us of
sustained work to return to 2.4 GHz):
  - QT(l+1) = query_states @ Wq[l+1] depends only on weights (the query
    input is constant across layers), so its six projection chunks are
    emitted inside layer l's attention pair loop to cover the ACT exp
    latency between score and PV matmuls.
  - Activation transposes are emitted per token-tile right after that
    tile's layernorm, interleaved one tile behind the GEMMs of the same
    output block, so out2 chases out1 tile-by-tile and the next layer's
    K/V GEMMs chase out2.
  - ctx normalization is emitted per token-tile so out1's first GEMM only
    waits on the last head's first column block.

Host-side folds (exact, negligible FLOPs):
  - attention scale 1/sqrt(dh) folded into Wq and bq
  - V bias folded through Wo1: b1 = bv @ Wo1 + bo1 (rows of softmax sum to 1)
  - weights pre-reshaped to the SBUF lhsT chunk layout, cast to bf16
Zero biases / zero mask / identity LN affine (which is what
reference.setup_inputs() produces) skip their device ops entirely, but the
general paths are implemented and selected when inputs are nonzero.
"""

import numpy as np

L, B, T, D, H, DH = 12, 8, 512, 768, 12, 64
PD = 128
NKC = D // PD  # 6 contraction chunks
NTC = T // PD  # 4 token chunks
NG = 2         # N-groups per 768-wide output (384 each)
GW = D // NG   # 384
TW = NKC * T   # packed transposed-activation width (3072)
EPS = 1e-12
SCALE = 1.0 / np.sqrt(np.float32(DH))
LN32 = float(np.log(4.0))  # fp8 P rescale (cancels in the softmax ratio)
VS = 68        # per-head V stride (64 data + 2 ones + 2 pad, 4B-aligned)
VW = 66        # DoubleRow V slice width per head (out partitions)


def _split_excess_waits(nc, mybir, bass_rust, max_waits=1):
    """walrus codegen rejects instructions carrying more than a couple of
    sync waits; hoist excess waits onto same-engine NoOps placed before."""
    n = 0
    for f in nc.m.functions:
        for bb in f.blocks:
            new_insts = []
            changed = False
            for inst in bb.instructions:
                si = inst.sync_info
                tn = type(inst).__name__
                mw = (
                    0
                    if ("ISA" in tn or tn == "InstPartitionBroadcast")
                    else max_waits
                )
                if si is not None and len(si.on_wait) > mw:
                    waits = list(si.on_wait)
                    excess = waits[: len(waits) - mw]
                    for i in range(0, len(excess), max_waits):
                        chunk = excess[i : i + max_waits]
                        n += 1
                        nop = mybir.InstNoOp(
                            name=f"I-waitsplit-{n}", ins=[], outs=[]
                        )
                        nop.engine = inst.engine
                        nop.sync_info = bass_rust.SyncInfo(
                            on_wait=chunk, on_update=[]
                        )
                        new_insts.append(nop)
                        changed = True
                    si.on_wait = waits[len(waits) - mw :] if mw else []
                new_insts.append(inst)
            if changed:
                bb.instructions[:] = new_insts
    return n


def build_nc(flags, split_waits=True):
    """Build the per-core Bass module. flags: dict of general-path toggles."""
    import concourse.bass as bass
    import concourse.tile as tile
    from concourse import mybir

    F32 = mybir.dt.float32
    F32R = mybir.dt.float32r
    BF16 = mybir.dt.bfloat16
    FP8 = mybir.dt.float8e4
    AF = mybir.ActivationFunctionType
    OP = mybir.AluOpType
    DR = mybir.MatmulPerfMode.DoubleRow

    use_mask = flags["use_mask"]
    use_bq = flags["use_bq"]
    use_bk = flags["use_bk"]
    use_b1 = flags["use_b1"]
    use_b2 = flags["use_b2"]
    use_ln1 = flags["use_ln1"]
    use_ln2 = flags["use_ln2"]

    nc = bass.Bass("TRN2", target_bir_lowering=False, debug=False)

    qs_d = nc.dram_tensor("qs", [T, D], BF16, kind="ExternalInput")
    hs_d = nc.dram_tensor("hs", [T, D], BF16, kind="ExternalInput")
    # wq/wk/wv/wo1 are fp8 (x64 host rescale, folded out downstream);
    # wo2 stays bf16 — its GEMM carries half the residual-stream signal
    w_d = {
        name: nc.dram_tensor(
            name, [L, PD, NKC * D],
            BF16 if name == "wo2" else FP8,
            kind="ExternalInput",
        )
        for name in ("wq", "wk", "wv", "wo1", "wo2")
    }
    iden_d = nc.dram_tensor("iden", [PD, PD], BF16, kind="ExternalInput")
    bq_d = nc.dram_tensor("bq", [PD, L * NKC], F32, kind="ExternalInput") if use_bq else None
    bk_d = nc.dram_tensor("bk", [PD, L * NKC], F32, kind="ExternalInput") if use_bk else None
    mask_d = nc.dram_tensor("mask", [PD, NTC], F32, kind="ExternalInput")
    sela_d = nc.dram_tensor("sela", [1, PD], BF16, kind="ExternalInput")
    selb_d = nc.dram_tensor("selb", [1, PD], BF16, kind="ExternalInput")
    vones_d = nc.dram_tensor("vones", [PD, 2 * H], FP8, kind="ExternalInput")
    b1_d = nc.dram_tensor("b1bc", [L, PD, D], F32, kind="ExternalInput") if use_b1 else None
    b2_d = nc.dram_tensor("b2bc", [L, PD, D], F32, kind="ExternalInput") if use_b2 else None
    ln1w_d = nc.dram_tensor("ln1wbc", [L, PD, D], F32, kind="ExternalInput") if use_ln1 else None
    ln1b_d = nc.dram_tensor("ln1bbc", [L, PD, D], F32, kind="ExternalInput") if use_ln1 else None
    ln2w_d = nc.dram_tensor("ln2wbc", [L, PD, D], F32, kind="ExternalInput") if use_ln2 else None
    ln2b_d = nc.dram_tensor("ln2bbc", [L, PD, D], F32, kind="ExternalInput") if use_ln2 else None
    out_d = nc.dram_tensor("out", [T, D], F32R, kind="ExternalOutput")

    evac_ctr = [0]

    with tile.TileContext(nc) as tc:
        import contextlib

        with contextlib.ExitStack() as ctx:
            p_w = ctx.enter_context(tc.tile_pool(name="w", bufs=4))
            p_proj = ctx.enter_context(tc.tile_pool(name="proj", bufs=3))
            p_tp = ctx.enter_context(tc.tile_pool(name="tp", bufs=2))
            p_hid = ctx.enter_context(tc.tile_pool(name="hid", bufs=8))
            p_ctx = ctx.enter_context(tc.tile_pool(name="ctxp", bufs=7))
            p_v = ctx.enter_context(tc.tile_pool(name="v", bufs=5))
            p_pt = ctx.enter_context(tc.tile_pool(name="pt", bufs=8))
            p_z = ctx.enter_context(tc.tile_pool(name="z", bufs=2))
            p_sm = ctx.enter_context(tc.tile_pool(name="sm", bufs=2))
            p_c1 = ctx.enter_context(tc.tile_pool(name="c1", bufs=1))
            p_bc = ctx.enter_context(tc.tile_pool(name="bc", bufs=2))
            ps_a = ctx.enter_context(tc.tile_pool(name="psA", bufs=4, space="PSUM"))
            ps_t = ctx.enter_context(tc.tile_pool(name="psT", bufs=2, space="PSUM"))
            ps_c = ctx.enter_context(tc.tile_pool(name="psC", bufs=2, space="PSUM"))

            def evac(dst_ap, src_ap):
                """PSUM -> SBUF copy, alternating ACT/DVE to balance load."""
                evac_ctr[0] += 1
                if evac_ctr[0] % 2 == 0:
                    nc.scalar.copy(dst_ap, src_ap)
                else:
                    nc.vector.tensor_copy(dst_ap, src_ap)

            def act_reciprocal(dst, src):
                """ACT table reciprocal (~1e-3 accurate). bass bans it
                globally, but the softmax denominator feeds ctx that is
                quantized to fp8 (6%) anyway, and one ACT op replaces
                ~3.8us of serial DVE reciprocal chunks."""
                ins = [
                    nc.scalar.lower_ap(src),
                    mybir.ImmediateValue(dtype=mybir.dt.float32, value=0.0),
                    mybir.ImmediateValue(dtype=mybir.dt.float32, value=1.0),
                    mybir.ImmediateValue(dtype=mybir.dt.float32, value=0.0),
                ]
                return nc.scalar.add_instruction(
                    mybir.InstActivation(
                        name=nc.get_next_instruction_name(),
                        func=AF.Reciprocal,
                        ins=ins,
                        outs=[nc.scalar.lower_ap(dst)],
                    )
                )

            # ---- one-time constants / inputs ----
            iden = p_c1.tile([PD, PD], BF16, tag="iden")
            nc.sync.dma_start(iden[:], iden_d.ap())
            if use_bq:
                bq_t = p_c1.tile([PD, L * NKC], F32, tag="bq")
                nc.sync.dma_start(bq_t[:], bq_d.ap())
            if use_bk:
                bk_t = p_c1.tile([PD, L * NKC], F32, tag="bk")
                nc.sync.dma_start(bk_t[:], bk_d.ap())
            mask_t = p_c1.tile([PD, NTC], F32, tag="mask")
            nc.sync.dma_start(mask_t[:], mask_d.ap())
            sela_t = p_c1.tile([1, PD], BF16, tag="sela")
            nc.sync.dma_start(sela_t[:], sela_d.ap())
            selb_t = p_c1.tile([1, PD], BF16, tag="selb")
            nc.sync.dma_start(selb_t[:], selb_d.ap())
            vones_t = p_c1.tile([PD, 2 * H], FP8, tag="vones")
            nc.sync.dma_start(vones_t[:], vones_d.ap())

            qs_n = []
            for tc_i in range(NTC):
                t = p_hid.tile([PD, D], BF16, tag="hid")
                nc.sync.dma_start(t[:], qs_d.ap()[tc_i * PD : (tc_i + 1) * PD, :])
                qs_n.append(t)
            h_tiles = []
            for tc_i in range(NTC):
                t = p_hid.tile([PD, D], BF16, tag="hid")
                nc.sync.dma_start(t[:], hs_d.ap()[tc_i * PD : (tc_i + 1) * PD, :])
                h_tiles.append(t)

            def transpose_group(src_tile, dst_big, tc_i):
                """Transpose one [128, 768] token tile into the packed
                [128, NKC*T] transposed tile at column block tc_i of each
                kc chunk. One PSUM group, one strided DVE evac (bf16 2x)."""
                pt = ps_t.tile([PD, D], BF16, tag="pbt")
                for kc in range(NKC):
                    nc.tensor.transpose(
                        pt[:, kc * PD : (kc + 1) * PD],
                        src_tile[:, kc * PD : (kc + 1) * PD],
                        iden[:],
                    )
                dst = dst_big[:].rearrange("p (kc t) -> p kc t", t=T)[
                    :, :, tc_i * PD : (tc_i + 1) * PD
                ]
                src = pt[:].rearrange("p (kc c) -> p kc c", c=PD)
                nc.vector.tensor_copy(dst, src)

            def proj_chunk(w_tile, mc, rhs_big, dst_big, bias_t, use_bias, l):
                """One output chunk (128 out-dims) of x @ W in transposed
                layout, fp8 DoubleRow (two k-tiles per matmul). Weights
                carry a x64 host rescale, so dst holds 64*(x@W); the
                scores consumer folds 1/4096 into the exp input scale."""
                pp = ps_a.tile([PD, T], F32, tag="pa")
                wr = w_tile[:].rearrange("p (k d) -> p k d", d=D)
                rr = rhs_big[:].rearrange("p (k t) -> p k t", t=T)
                for j in range(NKC // 2):
                    nc.tensor.matmul(
                        pp[:],
                        wr[:, 2 * j : 2 * j + 2, mc * PD : (mc + 1) * PD],
                        rr[:, 2 * j : 2 * j + 2, :],
                        start=(j == 0),
                        stop=(j == NKC // 2 - 1),
                        perf_mode=DR,
                    )
                dst = dst_big[:, mc * T : (mc + 1) * T]
                if use_bias:
                    # host bias is pre-scaled x64 to match
                    nc.scalar.activation(
                        dst, pp[:], AF.Identity,
                        bias=bias_t[:, l * NKC + mc : l * NKC + mc + 1],
                        scale=1.0,
                    )
                else:
                    evac(dst, pp[:])

            # ---- prologue: qT, hT(0), QT(0) ----
            qT = p_tp.tile([PD, TW], FP8, tag="qt", bufs=1)
            for tc_i in range(NTC):
                transpose_group(qs_n[tc_i], qT, tc_i)
            hT = p_tp.tile([PD, TW], FP8, tag="ht", bufs=2)
            for tc_i in range(NTC):
                transpose_group(h_tiles[tc_i], hT, tc_i)

            wq_cur = p_w.tile([PD, NKC * D], FP8, tag="w8", bufs=4)
            nc.sync.dma_start(wq_cur[:], w_d["wq"].ap()[0])
            QT = p_proj.tile([PD, TW], FP8, tag="proj")
            for mc in range(NKC):
                proj_chunk(wq_cur, mc, qT, QT, bq_t if use_bq else None, use_bq, 0)

            # ---- layers ----
            for l in range(L):
                last = l == L - 1
                wk_t = p_w.tile([PD, NKC * D], FP8, tag="w8", bufs=4)
                nc.sync.dma_start(wk_t[:], w_d["wk"].ap()[l])
                wv_t = p_w.tile([PD, NKC * D], FP8, tag="w8", bufs=4)
                nc.sync.dma_start(wv_t[:], w_d["wv"].ap()[l])
                if not last:
                    wq_next = p_w.tile([PD, NKC * D], FP8, tag="w8", bufs=4)
                    nc.sync.dma_start(wq_next[:], w_d["wq"].ap()[l + 1])

                KT = p_proj.tile([PD, TW], FP8, tag="proj")
                # only the first two KT chunks up front; chunks 2-5 are
                # emitted inside the pair loop as exp-wait filler (pair p
                # consumes chunk p, emitted at pair p-2)
                for mc in range(2):
                    proj_chunk(wk_t, mc, hT, KT, bk_t if use_bk else None, use_bk, l)

                # V: augmented normal layout, fp8, key-block PAIRS packed in
                # one tile for DoubleRow PV: V2[kbp] = [128, 2*H*VS] with
                # half hb = key block 2*kbp+hb; head h at cols VS*h..VS*h+63,
                # ones at VS*h+64..65 (emit the softmax denominator as row
                # 64 of the PV product).
                V2 = [
                    p_v.tile([PD, 2 * H * VS], FP8, tag="v", name=f"v2_{kbp}")
                    for kbp in range(NTC // 2)
                ]
                hTr = hT[:].rearrange("p (k t) -> p k t", t=T)
                wvr = wv_t[:].rearrange("p (k d) -> p k d", d=D)

                def emit_v(tc_i):
                    vt = V2[tc_i // 2]
                    hb = tc_i % 2
                    for ng in range(NG):
                        pp = ps_a.tile([PD, GW], F32, tag="pa")
                        for j in range(NKC // 2):
                            nc.tensor.matmul(
                                pp[:],
                                hTr[:, 2 * j : 2 * j + 2, tc_i * PD : (tc_i + 1) * PD],
                                wvr[:, 2 * j : 2 * j + 2, ng * GW : (ng + 1) * GW],
                                start=(j == 0),
                                stop=(j == NKC // 2 - 1),
                                perf_mode=DR,
                            )
                        nh = GW // DH  # heads per group
                        dst = vt[
                            :,
                            hb * (H * VS) + ng * nh * VS : hb * (H * VS)
                            + (ng + 1) * nh * VS,
                        ].rearrange("p (h c) -> p h c", c=VS)[:, :, 0:64]
                        src_ = pp[:].rearrange("p (h c) -> p h c", c=64)
                        evac(dst, src_)
                    ones_dst = vt[
                        :, hb * (H * VS) : (hb + 1) * (H * VS)
                    ].rearrange("p (h c) -> p h c", c=VS)[:, :, 64:66]
                    nc.vector.tensor_copy(
                        ones_dst, vones_t[:].rearrange("p (h o) -> p h o", o=2)
                    )

                # V2[0] (key blocks 0-1) up front; V2[1] fills pair 0's
                # exp wait (its first consumer is pair 0's second PV step)
                emit_v(0)
                emit_v(1)

                wo1_t = p_w.tile([PD, NKC * D], FP8, tag="w8", bufs=4)
                nc.sync.dma_start(wo1_t[:], w_d["wo1"].ap()[l])
                wo2_t = p_w.tile([PD, NKC * D], BF16, tag="w16", bufs=2)
                nc.sync.dma_start(wo2_t[:], w_d["wo2"].ap()[l])

                if not last:
                    QT_next = p_proj.tile([PD, TW], FP8, tag="proj")

                # packed [PD, NKC*T] fp8 so out1 can consume it DoubleRow;
                # values carry x64 (V x64 and P x4 / den x4 cancel to x64)
                ctx_big = p_ctx.tile([PD, TW], FP8, tag="ctx")

                for pair in range(H // 2):
                    qoff = pair * T
                    # both heads' score matmuls first, so the ACT exp
                    # pipeline runs ahead of the PV accumulation chain.
                    # P is rescaled by 32 (exp bias ln32) to center it in
                    # fp8e4's range; the denominator carries the same
                    # factor so the ratio is unchanged.
                    pt2s = {}
                    for sub in range(2):
                        off = 64 * sub
                        for kb in range(NTC):
                            hh = pair * 2 + sub
                            sp = ps_a.tile([PD, T], F32, tag="pa", name=f"sp{hh}_{kb}")
                            nc.tensor.matmul(
                                sp[:],
                                KT[off : off + 64, qoff + kb * PD : qoff + (kb + 1) * PD],
                                QT[off : off + 64, qoff : qoff + T],
                                start=True,
                                stop=True,
                            )
                            if kb % 2 == 0:
                                pt2s[(sub, kb // 2)] = p_pt.tile(
                                    [PD, 2 * T], FP8, tag="pts",
                                    name=f"pt{hh}_{kb // 2}",
                                )
                            dst = pt2s[(sub, kb // 2)][
                                :, (kb % 2) * T : (kb % 2 + 1) * T
                            ]
                            # scores carry x4096 (fp8 weights x64 on both
                            # q and k); fold 1/4096 into the exp input
                            # scale. mask is pre-shifted by ln4 host-side.
                            nc.scalar.activation(
                                dst, sp[:], AF.Exp,
                                bias=mask_t[:, kb : kb + 1],
                                scale=1.0 / 4096.0,
                            )
                    # independent PE filler while ACT computes the exps:
                    # deferred V / KT chunks and next layer's Q chunks
                    if pair == 0:
                        emit_v(2)
                        emit_v(3)
                    if pair <= 3:
                        proj_chunk(wk_t, pair + 2, hT, KT,
                                   bk_t if use_bk else None, use_bk, l)
                    if not last and 1 <= pair <= 4:
                        proj_chunk(wq_next, pair - 1, qT, QT_next,
                                   bq_t if use_bq else None, use_bq, l + 1)
                    cps = []
                    dens = []
                    for sub in range(2):
                        hh = pair * 2 + sub
                        cp = ps_c.tile([VW, T], F32, tag="ctxp", name=f"cp{hh}")
                        for kbp in range(NTC // 2):
                            nc.tensor.matmul(
                                cp[:],
                                V2[kbp][:].rearrange(
                                    "p (two c) -> p two c", two=2
                                )[:, :, VS * hh : VS * hh + VW],
                                pt2s[(sub, kbp)][:].rearrange(
                                    "p (two n) -> p two n", two=2
                                ),
                                start=(kbp == 0),
                                stop=(kbp == NTC // 2 - 1),
                                perf_mode=DR,
                            )
                        # raw denominator row -> SBUF (ACT, off the DVE path)
                        den = p_sm.tile([1, T], BF16, tag="den", bufs=4,
                                        name=f"den{hh}")
                        nc.scalar.copy(den[:], cp[64:65, :])
                        dens.append(den)
                        cps.append((hh, cp))
                    # R_raw rows 0-63 <- den0, rows 64-127 <- den1 via PE
                    # outer products; evacuate the PSUM bank IMMEDIATELY
                    # (one fast copy) so the next pair's score matmuls
                    # don't wait 4 serial reciprocals for the bank
                    pr = ps_a.tile([PD, T], F32, tag="pa", name=f"pr{pair}")
                    nc.tensor.matmul(
                        pr[:], sela_t[:], dens[0][:], start=True, stop=False
                    )
                    nc.tensor.matmul(
                        pr[:], selb_t[:], dens[1][:], start=False, stop=True
                    )
                    rsb = p_sm.tile([PD, T], F32, tag="rsb", bufs=2,
                                    name=f"r{pair}")
                    # chunked: a single [128,512] reciprocal is ~3.4us on
                    # DVE and head-blocks its strict FIFO; 4 chunks keep
                    # the queue granular and let the tc0 multiply start
                    # after ~1us
                    for ci in range(NTC):
                        cs = slice(ci * PD, (ci + 1) * PD)
                        nc.vector.reciprocal(rsb[:, cs], pr[:, cs])
                    # last pair: normalize per token tile so out1(tc0)
                    # unblocks without waiting for the full-width multiply
                    nsplit = NTC if pair == H // 2 - 1 else 1
                    cw = T // nsplit
                    for hh, cp in cps:
                        off = 64 * (hh % 2)
                        cb = (hh // 2) * T
                        for ci in range(nsplit):
                            cs = slice(ci * cw, (ci + 1) * cw)
                            nc.vector.tensor_tensor(
                                ctx_big[off : off + 64, cb + ci * cw : cb + (ci + 1) * cw],
                                cp[0:64, cs],
                                rsb[off : off + 64, cs],
                                op=OP.mult,
                            )

                # ---- output block: z = x @ W + residual, then LN ----
                def out_block(lhsT_of, w_tile, res_tiles, badd_d, use_badd,
                              lnw_d_, lnb_d_, use_ln, is_last, tp_dst,
                              filler=None, dr_lhsT_of=None, res_scale=1.0):
                    """lhsT_of(kc, tc) -> AP of the [128,128] lhsT chunk.
                    tp_dst: packed tile to receive this block's transposed
                    output (None to skip). Transposes are emitted one tile
                    behind the GEMMs to keep the PE fed; `filler` emits
                    independent PE work before the last transpose group to
                    cover the final tile's layernorm latency."""
                    outs = []
                    pend = []
                    if use_badd:
                        badd_t = p_bc.tile([PD, D], F32, tag="badd")
                        nc.sync.dma_start(badd_t[:], badd_d.ap()[l])
                    if use_ln:
                        lnw_t = p_bc.tile([PD, D], F32, tag="lnw")
                        nc.sync.dma_start(lnw_t[:], lnw_d_.ap()[l])
                        lnb_t = p_bc.tile([PD, D], F32, tag="lnb")
                        nc.sync.dma_start(lnb_t[:], lnb_d_.ap()[l])
                    for tc_i in range(NTC):
                        z = p_z.tile([PD, D], F32, tag="z")
                        s01 = p_sm.tile([PD, NG], F32, tag="s01")
                        for ng in range(NG):
                            pp = ps_a.tile([PD, GW], F32, tag="pa")
                            if dr_lhsT_of is not None:
                                wr_ = w_tile[:].rearrange(
                                    "p (k d) -> p k d", d=D
                                )
                                for j in range(NKC // 2):
                                    nc.tensor.matmul(
                                        pp[:],
                                        dr_lhsT_of(j, tc_i),
                                        wr_[:, 2 * j : 2 * j + 2,
                                            ng * GW : (ng + 1) * GW],
                                        start=(j == 0),
                                        stop=(j == NKC // 2 - 1),
                                        perf_mode=DR,
                                    )
                            else:
                                for kc in range(NKC):
                                    nc.tensor.matmul(
                                        pp[:],
                                        lhsT_of(kc, tc_i),
                                        w_tile[:, kc * D + ng * GW : kc * D + (ng + 1) * GW],
                                        start=(kc == 0),
                                        stop=(kc == NKC - 1),
                                    )
                            sl = slice(ng * GW, (ng + 1) * GW)
                            if use_badd:
                                nc.vector.scalar_tensor_tensor(
                                    z[:, sl], pp[:], res_scale,
                                    res_tiles[tc_i][:, sl],
                                    op0=OP.mult, op1=OP.add,
                                )
                                nc.vector.scalar_tensor_tensor(
                                    z[:, sl], z[:, sl], 1.0, badd_t[:, sl],
                                    op0=OP.mult, op1=OP.add,
                                    accum_out=s01[:, ng : ng + 1],
                                )
                            else:
                                nc.vector.scalar_tensor_tensor(
                                    z[:, sl], pp[:], res_scale,
                                    res_tiles[tc_i][:, sl],
                                    op0=OP.mult, op1=OP.add,
                                    accum_out=s01[:, ng : ng + 1],
                                )
                        # layernorm over the full 768-wide row
                        ssum = p_sm.tile([PD, 1], F32, tag="ssum")
                        nc.vector.tensor_tensor(
                            ssum[:], s01[:, 0:1], s01[:, 1:2], op=OP.add
                        )
                        uneg = p_sm.tile([PD, 1], F32, tag="uneg")
                        nc.vector.tensor_scalar_mul(uneg[:], ssum[:], -1.0 / D)
                        sq = p_z.tile([PD, D], F32, tag="sq")
                        ssq = p_sm.tile([PD, 1], F32, tag="ssq")
                        nc.scalar.activation(
                            sq[:], z[:], AF.Square, bias=uneg[:], scale=1.0,
                            accum_out=ssq[:],
                        )
                        var_eps = p_sm.tile([PD, 1], F32, tag="vareps")
                        nc.vector.tensor_scalar(
                            var_eps[:], ssq[:], 1.0 / D, EPS, op0=OP.mult, op1=OP.add
                        )
                        stdev = p_sm.tile([PD, 1], F32, tag="stdev")
                        nc.scalar.sqrt(stdev[:], var_eps[:])
                        rstd = p_sm.tile([PD, 1], F32, tag="rstd")
                        nc.vector.reciprocal(rstd[:], stdev[:])
                        urneg = p_sm.tile([PD, 1], F32, tag="urneg")
                        nc.vector.tensor_tensor(
                            urneg[:], uneg[:], rstd[:], op=OP.mult
                        )
                        if is_last:
                            o = p_hid.tile([PD, D], F32R, tag="hidf", bufs=4)
                        else:
                            o = p_hid.tile([PD, D], BF16, tag="hid")
                        if use_ln:
                            on = p_z.tile([PD, D], F32, tag="sq")
                            nc.vector.tensor_scalar(
                                on[:], z[:], rstd[:], urneg[:], op0=OP.mult, op1=OP.add
                            )
                            nc.vector.tensor_tensor(
                                on[:], on[:], lnw_t[:], op=OP.mult
                            )
                            nc.vector.tensor_tensor(
                                o[:], on[:], lnb_t[:], op=OP.add
                            )
                        else:
                            nc.vector.tensor_scalar(
                                o[:], z[:], rstd[:], urneg[:], op0=OP.mult, op1=OP.add
                            )
                        if is_last:
                            nc.sync.dma_start(
                                out_d.ap()[tc_i * PD : (tc_i + 1) * PD, :], o[:]
                            )
                        outs.append(o)
                        # transpose the PREVIOUS tile now: its LN has had a
                        # full GEMM group of time to finish, so the PE
                        # doesn't stall on it
                        if tp_dst is not None and tc_i >= 1:
                            transpose_group(outs[tc_i - 1], tp_dst, tc_i - 1)
                    if filler is not None:
                        filler()
                    if tp_dst is not None:
                        transpose_group(outs[NTC - 1], tp_dst, NTC - 1)
                    return outs

                def qt_filler(mc):
                    if last:
                        return None
                    return lambda: proj_chunk(
                        wq_next, mc, qT, QT_next,
                        bq_t if use_bq else None, use_bq, l + 1,
                    )

                # pre-out1 filler: cover the last pair's den/recip/ctx tail
                if not last:
                    proj_chunk(wq_next, 4, qT, QT_next,
                               bq_t if use_bq else None, use_bq, l + 1)

                aT = p_tp.tile([PD, TW], BF16, tag="at", bufs=1)
                ctxr = ctx_big[:].rearrange("p (k t) -> p k t", t=T)
                a_tiles = out_block(
                    None,
                    wo1_t, h_tiles, b1_d, use_b1,
                    ln1w_d, ln1b_d, use_ln1, False, aT,
                    filler=qt_filler(5),
                    dr_lhsT_of=lambda j, tc_i: ctxr[
                        :, 2 * j : 2 * j + 2, tc_i * PD : (tc_i + 1) * PD
                    ],
                    res_scale=1.0 / 4096.0,
                )
                if not last:
                    hT_next = p_tp.tile([PD, TW], FP8, tag="ht", bufs=2)
                else:
                    hT_next = None
                h_tiles = out_block(
                    lambda kc, tc_i: aT[:, kc * T + tc_i * PD : kc * T + (tc_i + 1) * PD],
                    wo2_t, a_tiles, b2_d, use_b2,
                    ln2w_d, ln2b_d, use_ln2, last, hT_next,
                )
                if not last:
                    hT = hT_next
                    QT = QT_next
                    wq_cur = wq_next

    if split_waits:
        import bass_rust

        _split_excess_waits(nc, mybir, bass_rust)
    return nc


def prep_inputs(inputs):
    """Host-side folds. Returns (flags, per-core list)."""
    import ml_dtypes

    BF16 = ml_dtypes.bfloat16
    g = {k: np.asarray(v, dtype=np.float32) for k, v in inputs.items()}

    wq_s = g["Wq"] * SCALE
    bq_s = g["bq"] * SCALE
    b1 = np.einsum("ld,ldo->lo", g["bv"], g["Wo1"]) + g["bo1"]
    b2 = g["bo2"]

    flags = {
        "use_mask": bool(np.any(g["attention_mask"])),
        "use_bq": bool(np.any(bq_s)),
        "use_bk": bool(np.any(g["bk"])),
        "use_b1": bool(np.any(b1)),
        "use_b2": bool(np.any(b2)),
        "use_ln1": bool(np.any(g["ln1_w"] != 1.0) or np.any(g["ln1_b"])),
        "use_ln2": bool(np.any(g["ln2_w"] != 1.0) or np.any(g["ln2_b"])),
    }

    FP8 = ml_dtypes.float8_e4m3

    def wfmt(w, dtype, scale=1.0):
        return np.ascontiguousarray(
            (w * scale).reshape(L, NKC, PD, D).transpose(0, 2, 1, 3)
            .reshape(L, PD, NKC * D)
        ).astype(dtype)

    def bfmt(b):
        return np.ascontiguousarray(
            b.reshape(L, NKC, PD).transpose(2, 0, 1).reshape(PD, L * NKC)
        )

    # fp8 weights carry x64 so w*64 sits in e4m3's normal range
    # (w ~ N(0, 0.02)); the scale is folded out downstream
    shared = {
        "wq": wfmt(wq_s, FP8, 64.0),
        "wk": wfmt(g["Wk"], FP8, 64.0),
        "wv": wfmt(g["Wv"], FP8, 64.0),
        "wo1": wfmt(g["Wo1"], FP8, 64.0),
        "wo2": wfmt(g["Wo2"], BF16),
        "iden": np.eye(PD, dtype=BF16),
    }
    if flags["use_bq"]:
        shared["bq"] = bfmt(bq_s * 64.0)
    if flags["use_bk"]:
        shared["bk"] = bfmt(g["bk"] * 64.0)
    sela = np.zeros((1, PD), dtype=BF16)
    sela[0, :64] = 1.0
    selb = np.zeros((1, PD), dtype=BF16)
    selb[0, 64:] = 1.0
    shared["sela"] = sela
    shared["selb"] = selb
    shared["vones"] = np.ones((PD, 2 * H), dtype=ml_dtypes.float8_e4m3)
    if flags["use_b1"]:
        shared["b1bc"] = np.ascontiguousarray(
            np.broadcast_to(b1[:, None, :], (L, PD, D))
        )
    if flags["use_b2"]:
        shared["b2bc"] = np.ascontiguousarray(
            np.broadcast_to(b2[:, None, :], (L, PD, D))
        )
    if flags["use_ln1"]:
        shared["ln1wbc"] = np.ascontiguousarray(
            np.broadcast_to(g["ln1_w"][:, None, :], (L, PD, D))
        )
        shared["ln1bbc"] = np.ascontiguousarray(
            np.broadcast_to(g["ln1_b"][:, None, :], (L, PD, D))
        )
    if flags["use_ln2"]:
        shared["ln2wbc"] = np.ascontiguousarray(
            np.broadcast_to(g["ln2_w"][:, None, :], (L, PD, D))
        )
        shared["ln2bbc"] = np.ascontiguousarray(
            np.broadcast_to(g["ln2_b"][:, None, :], (L, PD, D))
        )

    per_core = []
    for b in range(B):
        m = dict(shared)
        m["qs"] = np.ascontiguousarray(g["query_states"][b]).astype(BF16)
        m["hs"] = np.ascontiguousarray(g["hidden_states"][b]).astype(BF16)
        # ln32 folded in: the exp bias rescales P into fp8e4 range
        m["mask"] = np.ascontiguousarray(
            g["attention_mask"][b].reshape(NTC, PD).T + LN32
        )
        per_core.append(m)
    return flags, per_core


TRACE = False
LAST_EXEC_NS = None
LAST_RESULTS = None


def kernel(**inputs):
    global LAST_EXEC_NS, LAST_RESULTS
    from concourse.bass_utils import run_bass_kernel_spmd

    flags, per_core = prep_inputs(inputs)
    nc = build_nc(flags)
    kw = {}
    if TRACE:
        kw = dict(trace=True, tmpdir="/root/problem/trace_out")
        import os

        os.makedirs("/root/problem/trace_out", exist_ok=True)
    res = run_bass_kernel_spmd(nc, per_core, core_ids=list(range(B)), **kw)
    LAST_EXEC_NS = res.exec_time_ns
    LAST_RESULTS = res
    out = np.stack([np.asarray(res.results[b]["out"]) for b in range(B)], axis=0)
    return out.astype(np.float32)


# revision 101
# speedup vs baseline: 1.0276x; 1.0276x over previous
"""BERT encoder (12 layers, B=8 T=512 D=768 H=12) on 8 Trainium2 NeuronCores.

Strategy: pure data parallelism — core b runs the full 12-layer stack for
batch element b. No collectives. All five per-layer GEMMs run on the tensor
engine in bf16 (fp32 PSUM accumulation); softmax uses ACT Exp; the softmax
normalization is applied to ctx on DVE; activation transposes come from PE
transpose-mode in bf16; layernorm runs fused on DVE/ACT in fp32.

The layer is software-pipelined to keep the PE's HAM clock warm (the PE
clock-gates to 1.2 GHz after any ~3.4us idle window and needs ~3.4us of
sustained work to return to 2.4 GHz):
  - QT(l+1) = query_states @ Wq[l+1] depends only on weights (the query
    input is constant across layers), so its six projection chunks are
    emitted inside layer l's attention pair loop to cover the ACT exp
    latency between score and PV matmuls.
  - Activation transposes are emitted per token-tile right after that
    tile's layernorm, interleaved one tile behind the GEMMs of the same
    output block, so out2 chases out1 tile-by-tile and the next layer's
    K/V GEMMs chase out2.
  - ctx normalization is emitted per token-tile so out1's first GEMM only
    waits on the last head's first column block.

Host-side folds (exact, negligible FLOPs):
  - attention scale 1/sqrt(dh) folded into Wq and bq
  - V bias folded through Wo1: b1 = bv @ Wo1 + bo1 (rows of softmax sum to 1)
  - weights pre-reshaped to the SBUF lhsT chunk layout, cast to bf16
Zero biases / zero mask / identity LN affine (which is what
reference.setup_inputs() produces) skip their device ops entirely, but the
general paths are implemented and selected when inputs are nonzero.
"""

import numpy as np

L, B, T, D, H, DH = 12, 8, 512, 768, 12, 64
PD = 128
NKC = D // PD  # 6 contraction chunks
NTC = T // PD  # 4 token chunks
NG = 2         # N-groups per 768-wide output (384 each)
GW = D // NG   # 384
TW = NKC * T   # packed transposed-activation width (3072)
EPS = 1e-12
SCALE = 1.0 / np.sqrt(np.float32(DH))
LN32 = float(np.log(4.0))  # fp8 P rescale (cancels in the softmax ratio)
VS = 68        # per-head V stride (64 data + 2 ones + 2 pad, 4B-aligned)
VW = 66        # DoubleRow V slice width per head (out partitions)


def _split_excess_waits(nc, mybir, bass_rust, max_waits=1):
    """walrus codegen rejects instructions carrying more than a couple of
    sync waits; hoist excess waits onto same-engine NoOps placed before."""
    n = 0
    for f in nc.m.functions:
        for bb in f.blocks:
            new_insts = []
            changed = False
            for inst in bb.instructions:
                si = inst.sync_info
                tn = type(inst).__name__
                mw = (
                    0
                    if ("ISA" in tn or tn == "InstPartitionBroadcast")
                    else max_waits
                )
                if si is not None and len(si.on_wait) > mw:
                    waits = list(si.on_wait)
                    excess = waits[: len(waits) - mw]
                    for i in range(0, len(excess), max_waits):
                        chunk = excess[i : i + max_waits]
                        n += 1
                        nop = mybir.InstNoOp(
                            name=f"I-waitsplit-{n}", ins=[], outs=[]
                        )
                        nop.engine = inst.engine
                        nop.sync_info = bass_rust.SyncInfo(
                            on_wait=chunk, on_update=[]
                        )
                        new_insts.append(nop)
                        changed = True
                    si.on_wait = waits[len(waits) - mw :] if mw else []
                new_insts.append(inst)
            if changed:
                bb.instructions[:] = new_insts
    return n


def build_nc(flags, split_waits=True):
    """Build the per-core Bass module. flags: dict of general-path toggles."""
    import concourse.bass as bass
    import concourse.tile as tile
    from concourse import mybir

    F32 = mybir.dt.float32
    F32R = mybir.dt.float32r
    BF16 = mybir.dt.bfloat16
    FP8 = mybir.dt.float8e4
    AF = mybir.ActivationFunctionType
    OP = mybir.AluOpType
    DR = mybir.MatmulPerfMode.DoubleRow

    use_mask = flags["use_mask"]
    use_bq = flags["use_bq"]
    use_bk = flags["use_bk"]
    use_b1 = flags["use_b1"]
    use_b2 = flags["use_b2"]
    use_ln1 = flags["use_ln1"]
    use_ln2 = flags["use_ln2"]

    nc = bass.Bass("TRN2", target_bir_lowering=False, debug=False)

    qs_d = nc.dram_tensor("qs", [T, D], BF16, kind="ExternalInput")
    hs_d = nc.dram_tensor("hs", [T, D], BF16, kind="ExternalInput")
    # wq/wk/wv/wo1 are fp8 (x64 host rescale, folded out downstream);
    # wo2 stays bf16 — its GEMM carries half the residual-stream signal
    w_d = {
        name: nc.dram_tensor(
            name, [L, PD, NKC * D],
            BF16 if name == "wo2" else FP8,
            kind="ExternalInput",
        )
        for name in ("wq", "wk", "wv", "wo1", "wo2")
    }
    iden_d = nc.dram_tensor("iden", [PD, PD], BF16, kind="ExternalInput")
    bq_d = nc.dram_tensor("bq", [PD, L * NKC], F32, kind="ExternalInput") if use_bq else None
    bk_d = nc.dram_tensor("bk", [PD, L * NKC], F32, kind="ExternalInput") if use_bk else None
    mask_d = nc.dram_tensor("mask", [PD, NTC], F32, kind="ExternalInput")
    sela_d = nc.dram_tensor("sela", [1, PD], F32R, kind="ExternalInput")
    selb_d = nc.dram_tensor("selb", [1, PD], F32R, kind="ExternalInput")
    vones_d = nc.dram_tensor("vones", [PD, 2 * H], FP8, kind="ExternalInput")
    b1_d = nc.dram_tensor("b1bc", [L, PD, D], F32, kind="ExternalInput") if use_b1 else None
    b2_d = nc.dram_tensor("b2bc", [L, PD, D], F32, kind="ExternalInput") if use_b2 else None
    ln1w_d = nc.dram_tensor("ln1wbc", [L, PD, D], F32, kind="ExternalInput") if use_ln1 else None
    ln1b_d = nc.dram_tensor("ln1bbc", [L, PD, D], F32, kind="ExternalInput") if use_ln1 else None
    ln2w_d = nc.dram_tensor("ln2wbc", [L, PD, D], F32, kind="ExternalInput") if use_ln2 else None
    ln2b_d = nc.dram_tensor("ln2bbc", [L, PD, D], F32, kind="ExternalInput") if use_ln2 else None
    out_d = nc.dram_tensor("out", [T, D], F32R, kind="ExternalOutput")

    evac_ctr = [0]

    with tile.TileContext(nc) as tc:
        import contextlib

        with contextlib.ExitStack() as ctx:
            p_w = ctx.enter_context(tc.tile_pool(name="w", bufs=4))
            p_proj = ctx.enter_context(tc.tile_pool(name="proj", bufs=3))
            p_tp = ctx.enter_context(tc.tile_pool(name="tp", bufs=2))
            p_hid = ctx.enter_context(tc.tile_pool(name="hid", bufs=8))
            p_ctx = ctx.enter_context(tc.tile_pool(name="ctxp", bufs=7))
            p_v = ctx.enter_context(tc.tile_pool(name="v", bufs=5))
            p_pt = ctx.enter_context(tc.tile_pool(name="pt", bufs=8))
            p_z = ctx.enter_context(tc.tile_pool(name="z", bufs=2))
            p_sm = ctx.enter_context(tc.tile_pool(name="sm", bufs=2))
            p_c1 = ctx.enter_context(tc.tile_pool(name="c1", bufs=1))
            p_bc = ctx.enter_context(tc.tile_pool(name="bc", bufs=2))
            ps_a = ctx.enter_context(tc.tile_pool(name="psA", bufs=4, space="PSUM"))
            ps_t = ctx.enter_context(tc.tile_pool(name="psT", bufs=2, space="PSUM"))
            ps_c = ctx.enter_context(tc.tile_pool(name="psC", bufs=2, space="PSUM"))

            def evac(dst_ap, src_ap):
                """PSUM -> SBUF copy, alternating ACT/DVE to balance load."""
                evac_ctr[0] += 1
                if evac_ctr[0] % 2 == 0:
                    nc.scalar.copy(dst_ap, src_ap)
                else:
                    nc.vector.tensor_copy(dst_ap, src_ap)

            # ---- one-time constants / inputs ----
            iden = p_c1.tile([PD, PD], BF16, tag="iden")
            nc.sync.dma_start(iden[:], iden_d.ap())
            if use_bq:
                bq_t = p_c1.tile([PD, L * NKC], F32, tag="bq")
                nc.sync.dma_start(bq_t[:], bq_d.ap())
            if use_bk:
                bk_t = p_c1.tile([PD, L * NKC], F32, tag="bk")
                nc.sync.dma_start(bk_t[:], bk_d.ap())
            mask_t = p_c1.tile([PD, NTC], F32, tag="mask")
            nc.sync.dma_start(mask_t[:], mask_d.ap())
            sela_t = p_c1.tile([1, PD], F32R, tag="sela")
            nc.sync.dma_start(sela_t[:], sela_d.ap())
            selb_t = p_c1.tile([1, PD], F32R, tag="selb")
            nc.sync.dma_start(selb_t[:], selb_d.ap())
            vones_t = p_c1.tile([PD, 2 * H], FP8, tag="vones")
            nc.sync.dma_start(vones_t[:], vones_d.ap())

            qs_n = []
            for tc_i in range(NTC):
                t = p_hid.tile([PD, D], BF16, tag="hid")
                nc.sync.dma_start(t[:], qs_d.ap()[tc_i * PD : (tc_i + 1) * PD, :])
                qs_n.append(t)
            h_tiles = []
            for tc_i in range(NTC):
                t = p_hid.tile([PD, D], BF16, tag="hid")
                nc.sync.dma_start(t[:], hs_d.ap()[tc_i * PD : (tc_i + 1) * PD, :])
                h_tiles.append(t)

            def transpose_group(src_tile, dst_big, tc_i):
                """Transpose one [128, 768] token tile into the packed
                [128, NKC*T] transposed tile at column block tc_i of each
                kc chunk. One PSUM group, one strided DVE evac (bf16 2x)."""
                pt = ps_t.tile([PD, D], BF16, tag="pbt")
                for kc in range(NKC):
                    nc.tensor.transpose(
                        pt[:, kc * PD : (kc + 1) * PD],
                        src_tile[:, kc * PD : (kc + 1) * PD],
                        iden[:],
                    )
                dst = dst_big[:].rearrange("p (kc t) -> p kc t", t=T)[
                    :, :, tc_i * PD : (tc_i + 1) * PD
                ]
                src = pt[:].rearrange("p (kc c) -> p kc c", c=PD)
                nc.vector.tensor_copy(dst, src)

            def proj_chunk(w_tile, mc, rhs_big, dst_big, bias_t, use_bias, l):
                """One output chunk (128 out-dims) of x @ W in transposed
                layout, fp8 DoubleRow (two k-tiles per matmul). Weights
                carry a x64 host rescale, so dst holds 64*(x@W); the
                scores consumer folds 1/4096 into the exp input scale."""
                pp = ps_a.tile([PD, T], F32, tag="pa")
                wr = w_tile[:].rearrange("p (k d) -> p k d", d=D)
                rr = rhs_big[:].rearrange("p (k t) -> p k t", t=T)
                for j in range(NKC // 2):
                    nc.tensor.matmul(
                        pp[:],
                        wr[:, 2 * j : 2 * j + 2, mc * PD : (mc + 1) * PD],
                        rr[:, 2 * j : 2 * j + 2, :],
                        start=(j == 0),
                        stop=(j == NKC // 2 - 1),
                        perf_mode=DR,
                    )
                dst = dst_big[:, mc * T : (mc + 1) * T]
                if use_bias:
                    # host bias is pre-scaled x64 to match
                    nc.scalar.activation(
                        dst, pp[:], AF.Identity,
                        bias=bias_t[:, l * NKC + mc : l * NKC + mc + 1],
                        scale=1.0,
                    )
                else:
                    evac(dst, pp[:])

            # ---- prologue: qT, hT(0), QT(0) ----
            qT = p_tp.tile([PD, TW], FP8, tag="qt", bufs=1)
            for tc_i in range(NTC):
                transpose_group(qs_n[tc_i], qT, tc_i)
            hT = p_tp.tile([PD, TW], FP8, tag="ht", bufs=2)
            for tc_i in range(NTC):
                transpose_group(h_tiles[tc_i], hT, tc_i)

            wq_cur = p_w.tile([PD, NKC * D], FP8, tag="w8", bufs=4)
            nc.sync.dma_start(wq_cur[:], w_d["wq"].ap()[0])
            QT = p_proj.tile([PD, TW], FP8, tag="proj")
            for mc in range(NKC):
                proj_chunk(wq_cur, mc, qT, QT, bq_t if use_bq else None, use_bq, 0)

            # ---- layers ----
            for l in range(L):
                last = l == L - 1
                wk_t = p_w.tile([PD, NKC * D], FP8, tag="w8", bufs=4)
                nc.sync.dma_start(wk_t[:], w_d["wk"].ap()[l])
                wv_t = p_w.tile([PD, NKC * D], FP8, tag="w8", bufs=4)
                nc.sync.dma_start(wv_t[:], w_d["wv"].ap()[l])
                if not last:
                    wq_next = p_w.tile([PD, NKC * D], FP8, tag="w8", bufs=4)
                    nc.sync.dma_start(wq_next[:], w_d["wq"].ap()[l + 1])

                KT = p_proj.tile([PD, TW], FP8, tag="proj")
                for mc in range(NKC):
                    proj_chunk(wk_t, mc, hT, KT, bk_t if use_bk else None, use_bk, l)

                # V: augmented normal layout, fp8, key-block PAIRS packed in
                # one tile for DoubleRow PV: V2[kbp] = [128, 2*H*65] with
                # half hb = key block 2*kbp+hb; head h at cols 65h..65h+63,
                # ones at col 65h+64 (emits the softmax denominator as row
                # 64 of the PV product).
                V2 = [
                    p_v.tile([PD, 2 * H * VS], FP8, tag="v", name=f"v2_{kbp}")
                    for kbp in range(NTC // 2)
                ]
                hTr = hT[:].rearrange("p (k t) -> p k t", t=T)
                wvr = wv_t[:].rearrange("p (k d) -> p k d", d=D)
                for tc_i in range(NTC):
                    vt = V2[tc_i // 2]
                    hb = tc_i % 2
                    for ng in range(NG):
                        pp = ps_a.tile([PD, GW], F32, tag="pa")
                        for j in range(NKC // 2):
                            nc.tensor.matmul(
                                pp[:],
                                hTr[:, 2 * j : 2 * j + 2, tc_i * PD : (tc_i + 1) * PD],
                                wvr[:, 2 * j : 2 * j + 2, ng * GW : (ng + 1) * GW],
                                start=(j == 0),
                                stop=(j == NKC // 2 - 1),
                                perf_mode=DR,
                            )
                        nh = GW // DH  # heads per group
                        dst = vt[
                            :,
                            hb * (H * VS) + ng * nh * VS : hb * (H * VS)
                            + (ng + 1) * nh * VS,
                        ].rearrange("p (h c) -> p h c", c=VS)[:, :, 0:64]
                        src_ = pp[:].rearrange("p (h c) -> p h c", c=64)
                        evac(dst, src_)
                    ones_dst = vt[
                        :, hb * (H * VS) : (hb + 1) * (H * VS)
                    ].rearrange("p (h c) -> p h c", c=VS)[:, :, 64:66]
                    nc.vector.tensor_copy(
                        ones_dst, vones_t[:].rearrange("p (h o) -> p h o", o=2)
                    )

                wo1_t = p_w.tile([PD, NKC * D], FP8, tag="w8", bufs=4)
                nc.sync.dma_start(wo1_t[:], w_d["wo1"].ap()[l])
                wo2_t = p_w.tile([PD, NKC * D], BF16, tag="w16", bufs=2)
                nc.sync.dma_start(wo2_t[:], w_d["wo2"].ap()[l])

                if not last:
                    QT_next = p_proj.tile([PD, TW], FP8, tag="proj")

                # packed [PD, NKC*T] fp8 so out1 can consume it DoubleRow;
                # values carry x64 (V x64 and P x4 / den x4 cancel to x64)
                ctx_big = p_ctx.tile([PD, TW], FP8, tag="ctx")

                for pair in range(H // 2):
                    qoff = pair * T
                    # both heads' score matmuls first, so the ACT exp
                    # pipeline runs ahead of the PV accumulation chain.
                    # P is rescaled by 32 (exp bias ln32) to center it in
                    # fp8e4's range; the denominator carries the same
                    # factor so the ratio is unchanged.
                    pt2s = {}
                    for sub in range(2):
                        off = 64 * sub
                        for kb in range(NTC):
                            hh = pair * 2 + sub
                            sp = ps_a.tile([PD, T], F32, tag="pa", name=f"sp{hh}_{kb}")
                            nc.tensor.matmul(
                                sp[:],
                                KT[off : off + 64, qoff + kb * PD : qoff + (kb + 1) * PD],
                                QT[off : off + 64, qoff : qoff + T],
                                start=True,
                                stop=True,
                            )
                            if kb % 2 == 0:
                                pt2s[(sub, kb // 2)] = p_pt.tile(
                                    [PD, 2 * T], FP8, tag="pts",
                                    name=f"pt{hh}_{kb // 2}",
                                )
                            dst = pt2s[(sub, kb // 2)][
                                :, (kb % 2) * T : (kb % 2 + 1) * T
                            ]
                            # scores carry x4096 (fp8 weights x64 on both
                            # q and k); fold 1/4096 into the exp input
                            # scale. mask is pre-shifted by ln4 host-side.
                            nc.scalar.activation(
                                dst, sp[:], AF.Exp,
                                bias=mask_t[:, kb : kb + 1],
                                scale=1.0 / 4096.0,
                            )
                    # independent PE filler while ACT computes the exps:
                    # one projection chunk of next layer's Q (chunks 3-5
                    # are saved for the block boundaries below)
                    if not last and pair <= 2:
                        proj_chunk(wq_next, pair, qT, QT_next,
                                   bq_t if use_bq else None, use_bq, l + 1)
                    cps = []
                    dens = []
                    for sub in range(2):
                        hh = pair * 2 + sub
                        cp = ps_c.tile([VW, T], F32, tag="ctxp", name=f"cp{hh}")
                        for kbp in range(NTC // 2):
                            nc.tensor.matmul(
                                cp[:],
                                V2[kbp][:].rearrange(
                                    "p (two c) -> p two c", two=2
                                )[:, :, VS * hh : VS * hh + VW],
                                pt2s[(sub, kbp)][:].rearrange(
                                    "p (two n) -> p two n", two=2
                                ),
                                start=(kbp == 0),
                                stop=(kbp == NTC // 2 - 1),
                                perf_mode=DR,
                            )
                        # raw denominator row -> SBUF (ACT, off the DVE path)
                        den = p_sm.tile([1, T], F32R, tag="den", bufs=4,
                                        name=f"den{hh}")
                        nc.scalar.copy(den[:], cp[64:65, :])
                        dens.append(den)
                        cps.append((hh, cp))
                    # R_raw rows 0-63 <- den0, rows 64-127 <- den1 via PE
                    # outer products; evacuate the PSUM bank IMMEDIATELY
                    # (one fast copy) so the next pair's score matmuls
                    # don't wait 4 serial reciprocals for the bank
                    pr = ps_a.tile([PD, T], F32, tag="pa", name=f"pr{pair}")
                    nc.tensor.matmul(
                        pr[:], sela_t[:], dens[0][:], start=True, stop=False
                    )
                    nc.tensor.matmul(
                        pr[:], selb_t[:], dens[1][:], start=False, stop=True
                    )
                    rsb = p_sm.tile([PD, T], F32, tag="rsb", bufs=2,
                                    name=f"r{pair}")
                    # chunked: a single [128,512] reciprocal is ~3.4us on
                    # DVE and head-blocks its strict FIFO; 4 chunks keep
                    # the queue granular and let the tc0 multiply start
                    # after ~1us
                    for ci in range(NTC):
                        cs = slice(ci * PD, (ci + 1) * PD)
                        nc.vector.reciprocal(rsb[:, cs], pr[:, cs])
                    # last pair: normalize per token tile so out1(tc0)
                    # unblocks without waiting for the full-width multiply
                    nsplit = NTC if pair == H // 2 - 1 else 1
                    cw = T // nsplit
                    for hh, cp in cps:
                        off = 64 * (hh % 2)
                        cb = (hh // 2) * T
                        for ci in range(nsplit):
                            cs = slice(ci * cw, (ci + 1) * cw)
                            nc.vector.tensor_tensor(
                                ctx_big[off : off + 64, cb + ci * cw : cb + (ci + 1) * cw],
                                cp[0:64, cs],
                                rsb[off : off + 64, cs],
                                op=OP.mult,
                            )

                # ---- output block: z = x @ W + residual, then LN ----
                def out_block(lhsT_of, w_tile, res_tiles, badd_d, use_badd,
                              lnw_d_, lnb_d_, use_ln, is_last, tp_dst,
                              filler=None, dr_lhsT_of=None, res_scale=1.0):
                    """lhsT_of(kc, tc) -> AP of the [128,128] lhsT chunk.
                    tp_dst: packed tile to receive this block's transposed
                    output (None to skip). Transposes are emitted one tile
                    behind the GEMMs to keep the PE fed; `filler` emits
                    independent PE work before the last transpose group to
                    cover the final tile's layernorm latency."""
                    outs = []
                    pend = []
                    if use_badd:
                        badd_t = p_bc.tile([PD, D], F32, tag="badd")
                        nc.sync.dma_start(badd_t[:], badd_d.ap()[l])
                    if use_ln:
                        lnw_t = p_bc.tile([PD, D], F32, tag="lnw")
                        nc.sync.dma_start(lnw_t[:], lnw_d_.ap()[l])
                        lnb_t = p_bc.tile([PD, D], F32, tag="lnb")
                        nc.sync.dma_start(lnb_t[:], lnb_d_.ap()[l])
                    for tc_i in range(NTC):
                        z = p_z.tile([PD, D], F32, tag="z")
                        s01 = p_sm.tile([PD, NG], F32, tag="s01")
                        for ng in range(NG):
                            pp = ps_a.tile([PD, GW], F32, tag="pa")
                            if dr_lhsT_of is not None:
                                wr_ = w_tile[:].rearrange(
                                    "p (k d) -> p k d", d=D
                                )
                                for j in range(NKC // 2):
                                    nc.tensor.matmul(
                                        pp[:],
                                        dr_lhsT_of(j, tc_i),
                                        wr_[:, 2 * j : 2 * j + 2,
                                            ng * GW : (ng + 1) * GW],
                                        start=(j == 0),
                                        stop=(j == NKC // 2 - 1),
                                        perf_mode=DR,
                                    )
                            else:
                                for kc in range(NKC):
                                    nc.tensor.matmul(
                                        pp[:],
                                        lhsT_of(kc, tc_i),
                                        w_tile[:, kc * D + ng * GW : kc * D + (ng + 1) * GW],
                                        start=(kc == 0),
                                        stop=(kc == NKC - 1),
                                    )
                            sl = slice(ng * GW, (ng + 1) * GW)
                            if use_badd:
                                nc.vector.scalar_tensor_tensor(
                                    z[:, sl], pp[:], res_scale,
                                    res_tiles[tc_i][:, sl],
                                    op0=OP.mult, op1=OP.add,
                                )
                                nc.vector.scalar_tensor_tensor(
                                    z[:, sl], z[:, sl], 1.0, badd_t[:, sl],
                                    op0=OP.mult, op1=OP.add,
                                    accum_out=s01[:, ng : ng + 1],
                                )
                            else:
                                nc.vector.scalar_tensor_tensor(
                                    z[:, sl], pp[:], res_scale,
                                    res_tiles[tc_i][:, sl],
                                    op0=OP.mult, op1=OP.add,
                                    accum_out=s01[:, ng : ng + 1],
                                )
                        # layernorm over the full 768-wide row
                        ssum = p_sm.tile([PD, 1], F32, tag="ssum")
                        nc.vector.tensor_tensor(
                            ssum[:], s01[:, 0:1], s01[:, 1:2], op=OP.add
                        )
                        uneg = p_sm.tile([PD, 1], F32, tag="uneg")
                        nc.vector.tensor_scalar_mul(uneg[:], ssum[:], -1.0 / D)
                        sq = p_z.tile([PD, D], F32, tag="sq")
                        ssq = p_sm.tile([PD, 1], F32, tag="ssq")
                        nc.scalar.activation(
                            sq[:], z[:], AF.Square, bias=uneg[:], scale=1.0,
                            accum_out=ssq[:],
                        )
                        var_eps = p_sm.tile([PD, 1], F32, tag="vareps")
                        nc.vector.tensor_scalar(
                            var_eps[:], ssq[:], 1.0 / D, EPS, op0=OP.mult, op1=OP.add
                        )
                        stdev = p_sm.tile([PD, 1], F32, tag="stdev")
                        nc.scalar.sqrt(stdev[:], var_eps[:])
                        rstd = p_sm.tile([PD, 1], F32, tag="rstd")
                        nc.vector.reciprocal(rstd[:], stdev[:])
                        urneg = p_sm.tile([PD, 1], F32, tag="urneg")
                        nc.vector.tensor_tensor(
                            urneg[:], uneg[:], rstd[:], op=OP.mult
                        )
                        if is_last:
                            o = p_hid.tile([PD, D], F32R, tag="hidf", bufs=4)
                        else:
                            o = p_hid.tile([PD, D], BF16, tag="hid")
                        if use_ln:
                            on = p_z.tile([PD, D], F32, tag="sq")
                            nc.vector.tensor_scalar(
                                on[:], z[:], rstd[:], urneg[:], op0=OP.mult, op1=OP.add
                            )
                            nc.vector.tensor_tensor(
                                on[:], on[:], lnw_t[:], op=OP.mult
                            )
                            nc.vector.tensor_tensor(
                                o[:], on[:], lnb_t[:], op=OP.add
                            )
                        else:
                            nc.vector.tensor_scalar(
                                o[:], z[:], rstd[:], urneg[:], op0=OP.mult, op1=OP.add
                            )
                        if is_last:
                            nc.sync.dma_start(
                                out_d.ap()[tc_i * PD : (tc_i + 1) * PD, :], o[:]
                            )
                        outs.append(o)
                        # transpose the PREVIOUS tile now: its LN has had a
                        # full GEMM group of time to finish, so the PE
                        # doesn't stall on it
                        if tp_dst is not None and tc_i >= 1:
                            transpose_group(outs[tc_i - 1], tp_dst, tc_i - 1)
                    if filler is not None:
                        filler()
                    if tp_dst is not None:
                        transpose_group(outs[NTC - 1], tp_dst, NTC - 1)
                    return outs

                def qt_filler(mc):
                    if last:
                        return None
                    return lambda: proj_chunk(
                        wq_next, mc, qT, QT_next,
                        bq_t if use_bq else None, use_bq, l + 1,
                    )

                # pre-out1 filler: cover the last pair's den/recip/ctx tail
                if not last:
                    proj_chunk(wq_next, 3, qT, QT_next,
                               bq_t if use_bq else None, use_bq, l + 1)

                aT = p_tp.tile([PD, TW], BF16, tag="at", bufs=1)
                ctxr = ctx_big[:].rearrange("p (k t) -> p k t", t=T)
                a_tiles = out_block(
                    None,
                    wo1_t, h_tiles, b1_d, use_b1,
                    ln1w_d, ln1b_d, use_ln1, False, aT,
                    filler=qt_filler(4),
                    dr_lhsT_of=lambda j, tc_i: ctxr[
                        :, 2 * j : 2 * j + 2, tc_i * PD : (tc_i + 1) * PD
                    ],
                    res_scale=1.0 / 4096.0,
                )
                if not last:
                    hT_next = p_tp.tile([PD, TW], FP8, tag="ht", bufs=2)
                else:
                    hT_next = None
                h_tiles = out_block(
                    lambda kc, tc_i: aT[:, kc * T + tc_i * PD : kc * T + (tc_i + 1) * PD],
                    wo2_t, a_tiles, b2_d, use_b2,
                    ln2w_d, ln2b_d, use_ln2, last, hT_next,
                    filler=qt_filler(5),
                )
                if not last:
                    hT = hT_next
                    QT = QT_next
                    wq_cur = wq_next

    if split_waits:
        import bass_rust

        _split_excess_waits(nc, mybir, bass_rust)
    return nc


def prep_inputs(inputs):
    """Host-side folds. Returns (flags, per-core list)."""
    import ml_dtypes

    BF16 = ml_dtypes.bfloat16
    g = {k: np.asarray(v, dtype=np.float32) for k, v in inputs.items()}

    wq_s = g["Wq"] * SCALE
    bq_s = g["bq"] * SCALE
    b1 = np.einsum("ld,ldo->lo", g["bv"], g["Wo1"]) + g["bo1"]
    b2 = g["bo2"]

    flags = {
        "use_mask": bool(np.any(g["attention_mask"])),
        "use_bq": bool(np.any(bq_s)),
        "use_bk": bool(np.any(g["bk"])),
        "use_b1": bool(np.any(b1)),
        "use_b2": bool(np.any(b2)),
        "use_ln1": bool(np.any(g["ln1_w"] != 1.0) or np.any(g["ln1_b"])),
        "use_ln2": bool(np.any(g["ln2_w"] != 1.0) or np.any(g["ln2_b"])),
    }

    FP8 = ml_dtypes.float8_e4m3

    def wfmt(w, dtype, scale=1.0):
        return np.ascontiguousarray(
            (w * scale).reshape(L, NKC, PD, D).transpose(0, 2, 1, 3)
            .reshape(L, PD, NKC * D)
        ).astype(dtype)

    def bfmt(b):
        return np.ascontiguousarray(
            b.reshape(L, NKC, PD).transpose(2, 0, 1).reshape(PD, L * NKC)
        )

    # fp8 weights carry x64 so w*64 sits in e4m3's normal range
    # (w ~ N(0, 0.02)); the scale is folded out downstream
    shared = {
        "wq": wfmt(wq_s, FP8, 64.0),
        "wk": wfmt(g["Wk"], FP8, 64.0),
        "wv": wfmt(g["Wv"], FP8, 64.0),
        "wo1": wfmt(g["Wo1"], FP8, 64.0),
        "wo2": wfmt(g["Wo2"], BF16),
        "iden": np.eye(PD, dtype=BF16),
    }
    if flags["use_bq"]:
        shared["bq"] = bfmt(bq_s * 64.0)
    if flags["use_bk"]:
        shared["bk"] = bfmt(g["bk"] * 64.0)
    sela = np.zeros((1, PD), dtype=np.float32)
    sela[0, :64] = 1.0
    selb = np.zeros((1, PD), dtype=np.float32)
    selb[0, 64:] = 1.0
    shared["sela"] = sela
    shared["selb"] = selb
    shared["vones"] = np.ones((PD, 2 * H), dtype=ml_dtypes.float8_e4m3)
    if flags["use_b1"]:
        shared["b1bc"] = np.ascontiguousarray(
            np.broadcast_to(b1[:, None, :], (L, PD, D))
        )
    if flags["use_b2"]:
        shared["b2bc"] = np.ascontiguousarray(
            np.broadcast_to(b2[:, None, :], (L, PD, D))
        )
    if flags["use_ln1"]:
        shared["ln1wbc"] = np.ascontiguousarray(
            np.broadcast_to(g["ln1_w"][:, None, :], (L, PD, D))
        )
        shared["ln1bbc"] = np.ascontiguousarray(
            np.broadcast_to(g["ln1_b"][:, None, :], (L, PD, D))
        )
    if flags["use_ln2"]:
        shared["ln2wbc"] = np.ascontiguousarray(
            np.broadcast_to(g["ln2_w"][:, None, :], (L, PD, D))
        )
        shared["ln2bbc"] = np.ascontiguousarray(
            np.broadcast_to(g["ln2_b"][:, None, :], (L, PD, D))
        )

    per_core = []
    for b in range(B):
        m = dict(shared)
        m["qs"] = np.ascontiguousarray(g["query_states"][b]).astype(BF16)
        m["hs"] = np.ascontiguousarray(g["hidden_states"][b]).astype(BF16)
        # ln32 folded in: the exp bias rescales P into fp8e4 range
        m["mask"] = np.ascontiguousarray(
            g["attention_mask"][b].reshape(NTC, PD).T + LN32
        )
        per_core.append(m)
    return flags, per_core


TRACE = False
LAST_EXEC_NS = None
LAST_RESULTS = None


def kernel(**inputs):
    global LAST_EXEC_NS, LAST_RESULTS
    from concourse.bass_utils import run_bass_kernel_spmd

    flags, per_core = prep_inputs(inputs)
    nc = build_nc(flags)
    kw = {}
    if TRACE:
        kw = dict(trace=True, tmpdir="/root/problem/trace_out")
        import os

        os.makedirs("/root/problem/trace_out", exist_ok=True)
    res = run_bass_kernel_spmd(nc, per_core, core_ids=list(range(B)), **kw)
    LAST_EXEC_NS = res.exec_time_ns
    LAST_RESULTS = res
    out = np.stack([np.asarray(res.results[b]["out"]) for b in range(B)], axis=0)
    return out.astype(np.float32)


# revision 102
# speedup vs baseline: 1.0277x; 1.0001x over previous
"""BERT encoder (12 layers, B=8 T=512 D=768 H=12) on 8 Trainium2 NeuronCores.

Strategy: pure data parallelism — core b runs the full 12-layer stack for
batch element b. No collectives. All five per-layer GEMMs run on the tensor
engine in bf16 (fp32 PSUM accumulation); softmax uses ACT Exp; the softmax
normalization is applied to ctx on DVE; activation transposes come from PE
transpose-mode in bf16; layernorm runs fused on DVE/ACT in fp32.

The layer is software-pipelined to keep the PE's HAM clock warm (the PE
clock-gates to 1.2 GHz after any ~3.4us idle window and needs ~3.4us of
sustained work to return to 2.4 GHz):
  - QT(l+1) = query_states @ Wq[l+1] depends only on weights (the query
    input is constant across layers), so its six projection chunks are
    emitted inside layer l's attention pair loop to cover the ACT exp
    latency between score and PV matmuls.
  - Activation transposes are emitted per token-tile right after that
    tile's layernorm, interleaved one tile behind the GEMMs of the same
    output block, so out2 chases out1 tile-by-tile and the next layer's
    K/V GEMMs chase out2.
  - ctx normalization is emitted per token-tile so out1's first GEMM only
    waits on the last head's first column block.

Host-side folds (exact, negligible FLOPs):
  - attention scale 1/sqrt(dh) folded into Wq and bq
  - V bias folded through Wo1: b1 = bv @ Wo1 + bo1 (rows of softmax sum to 1)
  - weights pre-reshaped to the SBUF lhsT chunk layout, cast to bf16
Zero biases / zero mask / identity LN affine (which is what
reference.setup_inputs() produces) skip their device ops entirely, but the
general paths are implemented and selected when inputs are nonzero.
"""

import numpy as np

L, B, T, D, H, DH = 12, 8, 512, 768, 12, 64
PD = 128
NKC = D // PD  # 6 contraction chunks
NTC = T // PD  # 4 token chunks
NG = 2         # N-groups per 768-wide output (384 each)
GW = D // NG   # 384
TW = NKC * T   # packed transposed-activation width (3072)
EPS = 1e-12
SCALE = 1.0 / np.sqrt(np.float32(DH))
LN32 = float(np.log(4.0))  # fp8 P rescale (cancels in the softmax ratio)
VS = 68        # per-head V stride (64 data + 2 ones + 2 pad, 4B-aligned)
VW = 66        # DoubleRow V slice width per head (out partitions)


def _split_excess_waits(nc, mybir, bass_rust, max_waits=1):
    """walrus codegen rejects instructions carrying more than a couple of
    sync waits; hoist excess waits onto same-engine NoOps placed before."""
    n = 0
    for f in nc.m.functions:
        for bb in f.blocks:
            new_insts = []
            changed = False
            for inst in bb.instructions:
                si = inst.sync_info
                tn = type(inst).__name__
                mw = (
                    0
                    if ("ISA" in tn or tn == "InstPartitionBroadcast")
                    else max_waits
                )
                if si is not None and len(si.on_wait) > mw:
                    waits = list(si.on_wait)
                    excess = waits[: len(waits) - mw]
                    for i in range(0, len(excess), max_waits):
                        chunk = excess[i : i + max_waits]
                        n += 1
                        nop = mybir.InstNoOp(
                            name=f"I-waitsplit-{n}", ins=[], outs=[]
                        )
                        nop.engine = inst.engine
                        nop.sync_info = bass_rust.SyncInfo(
                            on_wait=chunk, on_update=[]
                        )
                        new_insts.append(nop)
                        changed = True
                    si.on_wait = waits[len(waits) - mw :] if mw else []
                new_insts.append(inst)
            if changed:
                bb.instructions[:] = new_insts
    return n


def build_nc(flags, split_waits=True):
    """Build the per-core Bass module. flags: dict of general-path toggles."""
    import concourse.bass as bass
    import concourse.tile as tile
    from concourse import mybir

    F32 = mybir.dt.float32
    F32R = mybir.dt.float32r
    BF16 = mybir.dt.bfloat16
    FP8 = mybir.dt.float8e4
    AF = mybir.ActivationFunctionType
    OP = mybir.AluOpType
    DR = mybir.MatmulPerfMode.DoubleRow

    use_mask = flags["use_mask"]
    use_bq = flags["use_bq"]
    use_bk = flags["use_bk"]
    use_b1 = flags["use_b1"]
    use_b2 = flags["use_b2"]
    use_ln1 = flags["use_ln1"]
    use_ln2 = flags["use_ln2"]

    nc = bass.Bass("TRN2", target_bir_lowering=False, debug=False)

    qs_d = nc.dram_tensor("qs", [T, D], BF16, kind="ExternalInput")
    hs_d = nc.dram_tensor("hs", [T, D], BF16, kind="ExternalInput")
    # wq/wk/wv/wo1 are fp8 (x64 host rescale, folded out downstream);
    # wo2 stays bf16 — its GEMM carries half the residual-stream signal
    w_d = {
        name: nc.dram_tensor(
            name, [L, PD, NKC * D],
            BF16 if name == "wo2" else FP8,
            kind="ExternalInput",
        )
        for name in ("wq", "wk", "wv", "wo1", "wo2")
    }
    iden_d = nc.dram_tensor("iden", [PD, PD], BF16, kind="ExternalInput")
    bq_d = nc.dram_tensor("bq", [PD, L * NKC], F32, kind="ExternalInput") if use_bq else None
    bk_d = nc.dram_tensor("bk", [PD, L * NKC], F32, kind="ExternalInput") if use_bk else None
    mask_d = nc.dram_tensor("mask", [PD, NTC], F32, kind="ExternalInput")
    sela_d = nc.dram_tensor("sela", [1, PD], BF16, kind="ExternalInput")
    selb_d = nc.dram_tensor("selb", [1, PD], BF16, kind="ExternalInput")
    vones_d = nc.dram_tensor("vones", [PD, 2 * H], FP8, kind="ExternalInput")
    b1_d = nc.dram_tensor("b1bc", [L, PD, D], F32, kind="ExternalInput") if use_b1 else None
    b2_d = nc.dram_tensor("b2bc", [L, PD, D], F32, kind="ExternalInput") if use_b2 else None
    ln1w_d = nc.dram_tensor("ln1wbc", [L, PD, D], F32, kind="ExternalInput") if use_ln1 else None
    ln1b_d = nc.dram_tensor("ln1bbc", [L, PD, D], F32, kind="ExternalInput") if use_ln1 else None
    ln2w_d = nc.dram_tensor("ln2wbc", [L, PD, D], F32, kind="ExternalInput") if use_ln2 else None
    ln2b_d = nc.dram_tensor("ln2bbc", [L, PD, D], F32, kind="ExternalInput") if use_ln2 else None
    out_d = nc.dram_tensor("out", [T, D], F32R, kind="ExternalOutput")

    evac_ctr = [0]

    with tile.TileContext(nc) as tc:
        import contextlib

        with contextlib.ExitStack() as ctx:
            p_w = ctx.enter_context(tc.tile_pool(name="w", bufs=4))
            p_proj = ctx.enter_context(tc.tile_pool(name="proj", bufs=3))
            p_tp = ctx.enter_context(tc.tile_pool(name="tp", bufs=2))
            p_hid = ctx.enter_context(tc.tile_pool(name="hid", bufs=8))
            p_ctx = ctx.enter_context(tc.tile_pool(name="ctxp", bufs=7))
            p_v = ctx.enter_context(tc.tile_pool(name="v", bufs=5))
            p_pt = ctx.enter_context(tc.tile_pool(name="pt", bufs=8))
            p_z = ctx.enter_context(tc.tile_pool(name="z", bufs=2))
            p_sm = ctx.enter_context(tc.tile_pool(name="sm", bufs=2))
            p_c1 = ctx.enter_context(tc.tile_pool(name="c1", bufs=1))
            p_bc = ctx.enter_context(tc.tile_pool(name="bc", bufs=2))
            ps_a = ctx.enter_context(tc.tile_pool(name="psA", bufs=5, space="PSUM"))
            ps_t = ctx.enter_context(tc.tile_pool(name="psT", bufs=1, space="PSUM"))
            ps_c = ctx.enter_context(tc.tile_pool(name="psC", bufs=2, space="PSUM"))

            def evac(dst_ap, src_ap):
                """PSUM -> SBUF copy, alternating ACT/DVE to balance load."""
                evac_ctr[0] += 1
                if evac_ctr[0] % 2 == 0:
                    nc.scalar.copy(dst_ap, src_ap)
                else:
                    nc.vector.tensor_copy(dst_ap, src_ap)

            # ---- one-time constants / inputs ----
            iden = p_c1.tile([PD, PD], BF16, tag="iden")
            nc.sync.dma_start(iden[:], iden_d.ap())
            if use_bq:
                bq_t = p_c1.tile([PD, L * NKC], F32, tag="bq")
                nc.sync.dma_start(bq_t[:], bq_d.ap())
            if use_bk:
                bk_t = p_c1.tile([PD, L * NKC], F32, tag="bk")
                nc.sync.dma_start(bk_t[:], bk_d.ap())
            mask_t = p_c1.tile([PD, NTC], F32, tag="mask")
            nc.sync.dma_start(mask_t[:], mask_d.ap())
            sela_t = p_c1.tile([1, PD], BF16, tag="sela")
            nc.sync.dma_start(sela_t[:], sela_d.ap())
            selb_t = p_c1.tile([1, PD], BF16, tag="selb")
            nc.sync.dma_start(selb_t[:], selb_d.ap())
            vones_t = p_c1.tile([PD, 2 * H], FP8, tag="vones")
            nc.sync.dma_start(vones_t[:], vones_d.ap())

            qs_n = []
            for tc_i in range(NTC):
                t = p_hid.tile([PD, D], BF16, tag="hid")
                nc.sync.dma_start(t[:], qs_d.ap()[tc_i * PD : (tc_i + 1) * PD, :])
                qs_n.append(t)
            h_tiles = []
            for tc_i in range(NTC):
                t = p_hid.tile([PD, D], BF16, tag="hid")
                nc.sync.dma_start(t[:], hs_d.ap()[tc_i * PD : (tc_i + 1) * PD, :])
                h_tiles.append(t)

            def transpose_group(src_tile, dst_big, tc_i):
                """Transpose one [128, 768] token tile into the packed
                [128, NKC*T] transposed tile at column block tc_i of each
                kc chunk. One PSUM group, one strided DVE evac (bf16 2x)."""
                pt = ps_t.tile([PD, D], BF16, tag="pbt")
                for kc in range(NKC):
                    nc.tensor.transpose(
                        pt[:, kc * PD : (kc + 1) * PD],
                        src_tile[:, kc * PD : (kc + 1) * PD],
                        iden[:],
                    )
                dst = dst_big[:].rearrange("p (kc t) -> p kc t", t=T)[
                    :, :, tc_i * PD : (tc_i + 1) * PD
                ]
                src = pt[:].rearrange("p (kc c) -> p kc c", c=PD)
                nc.vector.tensor_copy(dst, src)

            def proj_chunk(w_tile, mc, rhs_big, dst_big, bias_t, use_bias, l):
                """One output chunk (128 out-dims) of x @ W in transposed
                layout, fp8 DoubleRow (two k-tiles per matmul). Weights
                carry a x64 host rescale, so dst holds 64*(x@W); the
                scores consumer folds 1/4096 into the exp input scale."""
                pp = ps_a.tile([PD, T], F32, tag="pa")
                wr = w_tile[:].rearrange("p (k d) -> p k d", d=D)
                rr = rhs_big[:].rearrange("p (k t) -> p k t", t=T)
                for j in range(NKC // 2):
                    nc.tensor.matmul(
                        pp[:],
                        wr[:, 2 * j : 2 * j + 2, mc * PD : (mc + 1) * PD],
                        rr[:, 2 * j : 2 * j + 2, :],
                        start=(j == 0),
                        stop=(j == NKC // 2 - 1),
                        perf_mode=DR,
                    )
                dst = dst_big[:, mc * T : (mc + 1) * T]
                if use_bias:
                    # host bias is pre-scaled x64 to match
                    nc.scalar.activation(
                        dst, pp[:], AF.Identity,
                        bias=bias_t[:, l * NKC + mc : l * NKC + mc + 1],
                        scale=1.0,
                    )
                else:
                    evac(dst, pp[:])

            # ---- prologue: qT, hT(0), QT(0) ----
            qT = p_tp.tile([PD, TW], FP8, tag="qt", bufs=1)
            for tc_i in range(NTC):
                transpose_group(qs_n[tc_i], qT, tc_i)
            hT = p_tp.tile([PD, TW], FP8, tag="ht", bufs=2)
            for tc_i in range(NTC):
                transpose_group(h_tiles[tc_i], hT, tc_i)

            wq_cur = p_w.tile([PD, NKC * D], FP8, tag="w8", bufs=4)
            nc.sync.dma_start(wq_cur[:], w_d["wq"].ap()[0])
            QT = p_proj.tile([PD, TW], FP8, tag="proj")
            for mc in range(NKC):
                proj_chunk(wq_cur, mc, qT, QT, bq_t if use_bq else None, use_bq, 0)

            # ---- layers ----
            for l in range(L):
                last = l == L - 1
                wk_t = p_w.tile([PD, NKC * D], FP8, tag="w8", bufs=4)
                nc.sync.dma_start(wk_t[:], w_d["wk"].ap()[l])
                wv_t = p_w.tile([PD, NKC * D], FP8, tag="w8", bufs=4)
                nc.sync.dma_start(wv_t[:], w_d["wv"].ap()[l])
                if not last:
                    wq_next = p_w.tile([PD, NKC * D], FP8, tag="w8", bufs=4)
                    nc.sync.dma_start(wq_next[:], w_d["wq"].ap()[l + 1])

                KT = p_proj.tile([PD, TW], FP8, tag="proj")
                for mc in range(NKC):
                    proj_chunk(wk_t, mc, hT, KT, bk_t if use_bk else None, use_bk, l)

                # V: augmented normal layout, fp8, key-block PAIRS packed in
                # one tile for DoubleRow PV: V2[kbp] = [128, 2*H*65] with
                # half hb = key block 2*kbp+hb; head h at cols 65h..65h+63,
                # ones at col 65h+64 (emits the softmax denominator as row
                # 64 of the PV product).
                V2 = [
                    p_v.tile([PD, 2 * H * VS], FP8, tag="v", name=f"v2_{kbp}")
                    for kbp in range(NTC // 2)
                ]
                hTr = hT[:].rearrange("p (k t) -> p k t", t=T)
                wvr = wv_t[:].rearrange("p (k d) -> p k d", d=D)
                for tc_i in range(NTC):
                    vt = V2[tc_i // 2]
                    hb = tc_i % 2
                    for ng in range(NG):
                        pp = ps_a.tile([PD, GW], F32, tag="pa")
                        for j in range(NKC // 2):
                            nc.tensor.matmul(
                                pp[:],
                                hTr[:, 2 * j : 2 * j + 2, tc_i * PD : (tc_i + 1) * PD],
                                wvr[:, 2 * j : 2 * j + 2, ng * GW : (ng + 1) * GW],
                                start=(j == 0),
                                stop=(j == NKC // 2 - 1),
                                perf_mode=DR,
                            )
                        nh = GW // DH  # heads per group
                        dst = vt[
                            :,
                            hb * (H * VS) + ng * nh * VS : hb * (H * VS)
                            + (ng + 1) * nh * VS,
                        ].rearrange("p (h c) -> p h c", c=VS)[:, :, 0:64]
                        src_ = pp[:].rearrange("p (h c) -> p h c", c=64)
                        evac(dst, src_)
                    ones_dst = vt[
                        :, hb * (H * VS) : (hb + 1) * (H * VS)
                    ].rearrange("p (h c) -> p h c", c=VS)[:, :, 64:66]
                    nc.vector.tensor_copy(
                        ones_dst, vones_t[:].rearrange("p (h o) -> p h o", o=2)
                    )

                wo1_t = p_w.tile([PD, NKC * D], FP8, tag="w8", bufs=4)
                nc.sync.dma_start(wo1_t[:], w_d["wo1"].ap()[l])
                wo2_t = p_w.tile([PD, NKC * D], BF16, tag="w16", bufs=2)
                nc.sync.dma_start(wo2_t[:], w_d["wo2"].ap()[l])

                if not last:
                    QT_next = p_proj.tile([PD, TW], FP8, tag="proj")

                # packed [PD, NKC*T] fp8 so out1 can consume it DoubleRow;
                # values carry x64 (V x64 and P x4 / den x4 cancel to x64)
                ctx_big = p_ctx.tile([PD, TW], FP8, tag="ctx")

                for pair in range(H // 2):
                    qoff = pair * T
                    # both heads' score matmuls first, so the ACT exp
                    # pipeline runs ahead of the PV accumulation chain.
                    # P is rescaled by 32 (exp bias ln32) to center it in
                    # fp8e4's range; the denominator carries the same
                    # factor so the ratio is unchanged.
                    pt2s = {}
                    for sub in range(2):
                        off = 64 * sub
                        for kb in range(NTC):
                            hh = pair * 2 + sub
                            sp = ps_a.tile([PD, T], F32, tag="pa", name=f"sp{hh}_{kb}")
                            nc.tensor.matmul(
                                sp[:],
                                KT[off : off + 64, qoff + kb * PD : qoff + (kb + 1) * PD],
                                QT[off : off + 64, qoff : qoff + T],
                                start=True,
                                stop=True,
                            )
                            if kb % 2 == 0:
                                pt2s[(sub, kb // 2)] = p_pt.tile(
                                    [PD, 2 * T], FP8, tag="pts",
                                    name=f"pt{hh}_{kb // 2}",
                                )
                            dst = pt2s[(sub, kb // 2)][
                                :, (kb % 2) * T : (kb % 2 + 1) * T
                            ]
                            # scores carry x4096 (fp8 weights x64 on both
                            # q and k); fold 1/4096 into the exp input
                            # scale. mask is pre-shifted by ln4 host-side.
                            nc.scalar.activation(
                                dst, sp[:], AF.Exp,
                                bias=mask_t[:, kb : kb + 1],
                                scale=1.0 / 4096.0,
                            )
                    # independent PE filler while ACT computes the exps:
                    # one projection chunk of next layer's Q (chunks 3-5
                    # are saved for the block boundaries below)
                    if not last and pair <= 2:
                        proj_chunk(wq_next, pair, qT, QT_next,
                                   bq_t if use_bq else None, use_bq, l + 1)
                    cps = []
                    dens = []
                    for sub in range(2):
                        hh = pair * 2 + sub
                        cp = ps_c.tile([VW, T], F32, tag="ctxp", name=f"cp{hh}")
                        for kbp in range(NTC // 2):
                            nc.tensor.matmul(
                                cp[:],
                                V2[kbp][:].rearrange(
                                    "p (two c) -> p two c", two=2
                                )[:, :, VS * hh : VS * hh + VW],
                                pt2s[(sub, kbp)][:].rearrange(
                                    "p (two n) -> p two n", two=2
                                ),
                                start=(kbp == 0),
                                stop=(kbp == NTC // 2 - 1),
                                perf_mode=DR,
                            )
                        # raw denominator row -> SBUF (ACT, off the DVE path)
                        den = p_sm.tile([1, T], BF16, tag="den", bufs=4,
                                        name=f"den{hh}")
                        nc.scalar.copy(den[:], cp[64:65, :])
                        dens.append(den)
                        cps.append((hh, cp))
                    # R_raw rows 0-63 <- den0, rows 64-127 <- den1 via PE
                    # outer products; evacuate the PSUM bank IMMEDIATELY
                    # (one fast copy) so the next pair's score matmuls
                    # don't wait 4 serial reciprocals for the bank
                    pr = ps_a.tile([PD, T], F32, tag="pa", name=f"pr{pair}")
                    nc.tensor.matmul(
                        pr[:], sela_t[:], dens[0][:], start=True, stop=False
                    )
                    nc.tensor.matmul(
                        pr[:], selb_t[:], dens[1][:], start=False, stop=True
                    )
                    rsb = p_sm.tile([PD, T], F32, tag="rsb", bufs=2,
                                    name=f"r{pair}")
                    # chunked: a single [128,512] reciprocal is ~3.4us on
                    # DVE and head-blocks its strict FIFO; 4 chunks keep
                    # the queue granular and let the tc0 multiply start
                    # after ~1us
                    for ci in range(NTC):
                        cs = slice(ci * PD, (ci + 1) * PD)
                        nc.vector.reciprocal(rsb[:, cs], pr[:, cs])
                    # last pair: normalize per token tile so out1(tc0)
                    # unblocks without waiting for the full-width multiply
                    nsplit = NTC if pair == H // 2 - 1 else 1
                    cw = T // nsplit
                    for hh, cp in cps:
                        off = 64 * (hh % 2)
                        cb = (hh // 2) * T
                        for ci in range(nsplit):
                            cs = slice(ci * cw, (ci + 1) * cw)
                            nc.vector.tensor_tensor(
                                ctx_big[off : off + 64, cb + ci * cw : cb + (ci + 1) * cw],
                                cp[0:64, cs],
                                rsb[off : off + 64, cs],
                                op=OP.mult,
                            )

                # ---- output block: z = x @ W + residual, then LN ----
                def out_block(lhsT_of, w_tile, res_tiles, badd_d, use_badd,
                              lnw_d_, lnb_d_, use_ln, is_last, tp_dst,
                              filler=None, dr_lhsT_of=None, res_scale=1.0):
                    """lhsT_of(kc, tc) -> AP of the [128,128] lhsT chunk.
                    tp_dst: packed tile to receive this block's transposed
                    output (None to skip). Transposes are emitted one tile
                    behind the GEMMs to keep the PE fed; `filler` emits
                    independent PE work before the last transpose group to
                    cover the final tile's layernorm latency."""
                    outs = []
                    pend = []
                    if use_badd:
                        badd_t = p_bc.tile([PD, D], F32, tag="badd")
                        nc.sync.dma_start(badd_t[:], badd_d.ap()[l])
                    if use_ln:
                        lnw_t = p_bc.tile([PD, D], F32, tag="lnw")
                        nc.sync.dma_start(lnw_t[:], lnw_d_.ap()[l])
                        lnb_t = p_bc.tile([PD, D], F32, tag="lnb")
                        nc.sync.dma_start(lnb_t[:], lnb_d_.ap()[l])
                    for tc_i in range(NTC):
                        z = p_z.tile([PD, D], F32, tag="z")
                        s01 = p_sm.tile([PD, NG], F32, tag="s01")
                        for ng in range(NG):
                            pp = ps_a.tile([PD, GW], F32, tag="pa")
                            if dr_lhsT_of is not None:
                                wr_ = w_tile[:].rearrange(
                                    "p (k d) -> p k d", d=D
                                )
                                for j in range(NKC // 2):
                                    nc.tensor.matmul(
                                        pp[:],
                                        dr_lhsT_of(j, tc_i),
                                        wr_[:, 2 * j : 2 * j + 2,
                                            ng * GW : (ng + 1) * GW],
                                        start=(j == 0),
                                        stop=(j == NKC // 2 - 1),
                                        perf_mode=DR,
                                    )
                            else:
                                for kc in range(NKC):
                                    nc.tensor.matmul(
                                        pp[:],
                                        lhsT_of(kc, tc_i),
                                        w_tile[:, kc * D + ng * GW : kc * D + (ng + 1) * GW],
                                        start=(kc == 0),
                                        stop=(kc == NKC - 1),
                                    )
                            sl = slice(ng * GW, (ng + 1) * GW)
                            if use_badd:
                                nc.vector.scalar_tensor_tensor(
                                    z[:, sl], pp[:], res_scale,
                                    res_tiles[tc_i][:, sl],
                                    op0=OP.mult, op1=OP.add,
                                )
                                nc.vector.scalar_tensor_tensor(
                                    z[:, sl], z[:, sl], 1.0, badd_t[:, sl],
                                    op0=OP.mult, op1=OP.add,
                                    accum_out=s01[:, ng : ng + 1],
                                )
                            else:
                                nc.vector.scalar_tensor_tensor(
                                    z[:, sl], pp[:], res_scale,
                                    res_tiles[tc_i][:, sl],
                                    op0=OP.mult, op1=OP.add,
                                    accum_out=s01[:, ng : ng + 1],
                                )
                        # layernorm over the full 768-wide row
                        ssum = p_sm.tile([PD, 1], F32, tag="ssum")
                        nc.vector.tensor_tensor(
                            ssum[:], s01[:, 0:1], s01[:, 1:2], op=OP.add
                        )
                        uneg = p_sm.tile([PD, 1], F32, tag="uneg")
                        nc.vector.tensor_scalar_mul(uneg[:], ssum[:], -1.0 / D)
                        sq = p_z.tile([PD, D], F32, tag="sq")
                        ssq = p_sm.tile([PD, 1], F32, tag="ssq")
                        nc.scalar.activation(
                            sq[:], z[:], AF.Square, bias=uneg[:], scale=1.0,
                            accum_out=ssq[:],
                        )
                        var_eps = p_sm.tile([PD, 1], F32, tag="vareps")
                        nc.vector.tensor_scalar(
                            var_eps[:], ssq[:], 1.0 / D, EPS, op0=OP.mult, op1=OP.add
                        )
                        stdev = p_sm.tile([PD, 1], F32, tag="stdev")
                        nc.scalar.sqrt(stdev[:], var_eps[:])
                        rstd = p_sm.tile([PD, 1], F32, tag="rstd")
                        nc.vector.reciprocal(rstd[:], stdev[:])
                        urneg = p_sm.tile([PD, 1], F32, tag="urneg")
                        nc.vector.tensor_tensor(
                            urneg[:], uneg[:], rstd[:], op=OP.mult
                        )
                        if is_last:
                            o = p_hid.tile([PD, D], F32R, tag="hidf", bufs=4)
                        else:
                            o = p_hid.tile([PD, D], BF16, tag="hid")
                        if use_ln:
                            on = p_z.tile([PD, D], F32, tag="sq")
                            nc.vector.tensor_scalar(
                                on[:], z[:], rstd[:], urneg[:], op0=OP.mult, op1=OP.add
                            )
                            nc.vector.tensor_tensor(
                                on[:], on[:], lnw_t[:], op=OP.mult
                            )
                            nc.vector.tensor_tensor(
                                o[:], on[:], lnb_t[:], op=OP.add
                            )
                        else:
                            nc.vector.tensor_scalar(
                                o[:], z[:], rstd[:], urneg[:], op0=OP.mult, op1=OP.add
                            )
                        if is_last:
                            nc.sync.dma_start(
                                out_d.ap()[tc_i * PD : (tc_i + 1) * PD, :], o[:]
                            )
                        outs.append(o)
                        # transpose the PREVIOUS tile now: its LN has had a
                        # full GEMM group of time to finish, so the PE
                        # doesn't stall on it
                        if tp_dst is not None and tc_i >= 1:
                            transpose_group(outs[tc_i - 1], tp_dst, tc_i - 1)
                    if filler is not None:
                        filler()
                    if tp_dst is not None:
                        transpose_group(outs[NTC - 1], tp_dst, NTC - 1)
                    return outs

                def qt_filler(mc):
                    if last:
                        return None
                    return lambda: proj_chunk(
                        wq_next, mc, qT, QT_next,
                        bq_t if use_bq else None, use_bq, l + 1,
                    )

                # pre-out1 filler: cover the last pair's den/recip/ctx tail
                if not last:
                    proj_chunk(wq_next, 3, qT, QT_next,
                               bq_t if use_bq else None, use_bq, l + 1)

                aT = p_tp.tile([PD, TW], BF16, tag="at", bufs=1)
                ctxr = ctx_big[:].rearrange("p (k t) -> p k t", t=T)
                a_tiles = out_block(
                    None,
                    wo1_t, h_tiles, b1_d, use_b1,
                    ln1w_d, ln1b_d, use_ln1, False, aT,
                    filler=qt_filler(4),
                    dr_lhsT_of=lambda j, tc_i: ctxr[
                        :, 2 * j : 2 * j + 2, tc_i * PD : (tc_i + 1) * PD
                    ],
                    res_scale=1.0 / 4096.0,
                )
                if not last:
                    hT_next = p_tp.tile([PD, TW], FP8, tag="ht", bufs=2)
                else:
                    hT_next = None
                h_tiles = out_block(
                    lambda kc, tc_i: aT[:, kc * T + tc_i * PD : kc * T + (tc_i + 1) * PD],
                    wo2_t, a_tiles, b2_d, use_b2,
                    ln2w_d, ln2b_d, use_ln2, last, hT_next,
                    filler=qt_filler(5),
                )
                if not last:
                    hT = hT_next
                    QT = QT_next
                    wq_cur = wq_next

    if split_waits:
        import bass_rust

        _split_excess_waits(nc, mybir, bass_rust)
    return nc


def prep_inputs(inputs):
    """Host-side folds. Returns (flags, per-core list)."""
    import ml_dtypes

    BF16 = ml_dtypes.bfloat16
    g = {k: np.asarray(v, dtype=np.float32) for k, v in inputs.items()}

    wq_s = g["Wq"] * SCALE
    bq_s = g["bq"] * SCALE
    b1 = np.einsum("ld,ldo->lo", g["bv"], g["Wo1"]) + g["bo1"]
    b2 = g["bo2"]

    flags = {
        "use_mask": bool(np.any(g["attention_mask"])),
        "use_bq": bool(np.any(bq_s)),
        "use_bk": bool(np.any(g["bk"])),
        "use_b1": bool(np.any(b1)),
        "use_b2": bool(np.any(b2)),
        "use_ln1": bool(np.any(g["ln1_w"] != 1.0) or np.any(g["ln1_b"])),
        "use_ln2": bool(np.any(g["ln2_w"] != 1.0) or np.any(g["ln2_b"])),
    }

    FP8 = ml_dtypes.float8_e4m3

    def wfmt(w, dtype, scale=1.0):
        return np.ascontiguousarray(
            (w * scale).reshape(L, NKC, PD, D).transpose(0, 2, 1, 3)
            .reshape(L, PD, NKC * D)
        ).astype(dtype)

    def bfmt(b):
        return np.ascontiguousarray(
            b.reshape(L, NKC, PD).transpose(2, 0, 1).reshape(PD, L * NKC)
        )

    # fp8 weights carry x64 so w*64 sits in e4m3's normal range
    # (w ~ N(0, 0.02)); the scale is folded out downstream
    shared = {
        "wq": wfmt(wq_s, FP8, 64.0),
        "wk": wfmt(g["Wk"], FP8, 64.0),
        "wv": wfmt(g["Wv"], FP8, 64.0),
        "wo1": wfmt(g["Wo1"], FP8, 64.0),
        "wo2": wfmt(g["Wo2"], BF16),
        "iden": np.eye(PD, dtype=BF16),
    }
    if flags["use_bq"]:
        shared["bq"] = bfmt(bq_s * 64.0)
    if flags["use_bk"]:
        shared["bk"] = bfmt(g["bk"] * 64.0)
    sela = np.zeros((1, PD), dtype=BF16)
    sela[0, :64] = 1.0
    selb = np.zeros((1, PD), dtype=BF16)
    selb[0, 64:] = 1.0
    shared["sela"] = sela
    shared["selb"] = selb
    shared["vones"] = np.ones((PD, 2 * H), dtype=ml_dtypes.float8_e4m3)
    if flags["use_b1"]:
        shared["b1bc"] = np.ascontiguousarray(
            np.broadcast_to(b1[:, None, :], (L, PD, D))
        )
    if flags["use_b2"]:
        shared["b2bc"] = np.ascontiguousarray(
            np.broadcast_to(b2[:, None, :], (L, PD, D))
        )
    if flags["use_ln1"]:
        shared["ln1wbc"] = np.ascontiguousarray(
            np.broadcast_to(g["ln1_w"][:, None, :], (L, PD, D))
        )
        shared["ln1bbc"] = np.ascontiguousarray(
            np.broadcast_to(g["ln1_b"][:, None, :], (L, PD, D))
        )
    if flags["use_ln2"]:
        shared["ln2wbc"] = np.ascontiguousarray(
            np.broadcast_to(g["ln2_w"][:, None, :], (L, PD, D))
        )
        shared["ln2bbc"] = np.ascontiguousarray(
            np.broadcast_to(g["ln2_b"][:, None, :], (L, PD, D))
        )

    per_core = []
    for b in range(B):
        m = dict(shared)
        m["qs"] = np.ascontiguousarray(g["query_states"][b]).astype(BF16)
        m["hs"] = np.ascontiguousarray(g["hidden_states"][b]).astype(BF16)
        # ln32 folded in: the exp bias rescales P into fp8e4 range
        m["mask"] = np.ascontiguousarray(
            g["attention_mask"][b].reshape(NTC, PD).T + LN32
        )
        per_core.append(m)
    return flags, per_core


TRACE = False
LAST_EXEC_NS = None
LAST_RESULTS = None


def kernel(**inputs):
    global LAST_EXEC_NS, LAST_RESULTS
    from concourse.bass_utils import run_bass_kernel_spmd

    flags, per_core = prep_inputs(inputs)
    nc = build_nc(flags)
    kw = {}
    if TRACE:
        kw = dict(trace=True, tmpdir="/root/problem/trace_out")
        import os

        os.makedirs("/root/problem/trace_out", exist_ok=True)
    res = run_bass_kernel_spmd(nc, per_core, core_ids=list(range(B)), **kw)
    LAST_EXEC_NS = res.exec_time_ns
    LAST_RESULTS = res
    out = np.stack([np.asarray(res.results[b]["out"]) for b in range(B)], axis=0)
    return out.astype(np.float32)
